# revision 12
# baseline (speedup 1.0000x reference)
"""Trainium2 Bass kernel for the ConvNet+MLP GNN-message-passing module.

Pure data parallel over batch: 8 cores x 512 rows.

Per core:
  - conv1/conv2/fc1/base as Toeplitz matmuls in feature-on-partition layout
    (obs transposed once via PE transposes).
  - Neighbor slots processed per-a (20 slots x all 512 rows). The
    valid/expand/single selection is folded into additive penalty terms
    carried by the same matmuls that produce the per-slot hidden inputs,
    using relu(c+w) = max(c,-w)+w so penalized slots contribute exactly 0.
  - Expand sum over 21 actions: fused DVE ops acc=max(c,M-w_j)+acc, with
    some actions on ScalarE relu + TensorE identity-fold accumulate.
  - q = [H; n; 1]^T @ [fc3_w; fc3_b; const] per 128-row chunk.
"""

import numpy as np

B, A, ACT, ID, FEAT, HID, NA = 4096, 20, 21, 20, 32, 64, 21
VIEW = 210
OBS_D = 682
NCORES = 8
R = B // NCORES  # 512
M_PEN = 30.0
SENTINEL = -99.0

J_DVE = list(range(0, 13))   # expand actions on DVE stt path
J_ACT = list(range(13, 21))  # expand actions on ScalarE+PE path

_CACHE = {}


def _build_consts(conv1_w, conv1_b, conv2_w, conv2_b, fc1_w, fc1_b,
                  fc2_w, fc2_b, fc3_w, fc3_b):
    f32 = np.float32
    # conv1 Toeplitz [210, 640]: rows (c,h,w), cols (o,y,x)
    W1 = np.zeros((VIEW, 640), f32)
    for y in range(5):
        for x in range(4):
            for dy in range(3):
                for dx in range(3):
                    h, w = y + dy, x + dx
                    for c in range(5):
                        W1[c * 42 + h * 6 + w].reshape(32, 20)[:, y * 4 + x] += \
                            conv1_w[:, c, dy, dx]
    B1 = np.repeat(np.asarray(conv1_b, f32), 20)
    # conv2 Toeplitz [640, 192]: rows (o1,y1,x1), cols (o2,y2,x2)
    W2 = np.zeros((640, 192), f32)
    for y2 in range(3):
        for x2 in range(2):
            for dy in range(3):
                for dx in range(3):
                    y1, x1 = y2 + dy, x2 + dx
                    for o1 in range(32):
                        W2[o1 * 20 + y1 * 4 + x1].reshape(32, 6)[:, y2 * 2 + x2] += \
                            conv2_w[:, o1, dy, dx]
    B2 = np.repeat(np.asarray(conv2_b, f32), 6)

    fc2_w = np.asarray(fc2_w, f32)
    W2x = fc2_w[:HID]
    W2f = fc2_w[HID:HID + FEAT]
    W2id = fc2_w[HID + FEAT:HID + FEAT + ID]   # [20, 64]
    W2act = fc2_w[HID + FEAT + ID:]            # [21, 64]
    fc3_w = np.asarray(fc3_w, f32)

    W2IDP = np.zeros((21, HID), f32)  # row i <-> id == i-1 (row 0: self -> 0)
    W2IDP[1:] = W2id

    M = M_PEN
    # K-side lhsT weights replicated at the 4 32-row groups.
    # onehot slice rows: 32*al + i (i=0..20, id = i-1)
    # acts  slice rows: 32*al + e (e=0 ids row -> 0 weight, e=1+j -> action j)
    LOH_AB = np.zeros((128, 128), f32)
    LAC_AB = np.zeros((128, 128), f32)
    LOH_E = np.zeros((128, 64), f32)
    LAC_E = np.zeros((128, 64), f32)
    for al in range(4):
        r = 32 * al
        LOH_AB[r:r + 21, 0:64] = W2IDP + M          # A: idterm + M*(u+e)
        LOH_AB[r, 64:128] = M                        # B: M*e
        LAC_AB[r + 1:r + 22, 0:64] = W2act + M       # A: actterm + M*v
        LAC_AB[r + 1:r + 22, 64:128] = -M            # B: -M*v
        LOH_E[r + 1:r + 21, :] = W2id + M            # E: idterm + M*u
        LAC_E[r + 1:r + 22, :] = -M                  # E: -M*v
    LB_AB = np.concatenate([np.eye(HID, dtype=f32)] * 2, axis=1)  # [64,128]
    LB_E = np.eye(HID, dtype=f32)

    SCAL_AB = np.zeros((128, 1), f32)
    SCAL_AB[0:64] = 2 * M
    SCAL_AB[64:128] = M

    NWA = np.tile((M - W2act).T, (2, 1)).astype(f32)   # [128, 21]
    WAB = np.ascontiguousarray(-NWA)                    # ACT bias w_j - M

    # ids replication lhsT [128, 128]: out[32al+i] = in[32al] for i=0..20
    REPL = np.zeros((128, 128), f32)
    for al in range(4):
        REPL[32 * al, 32 * al:32 * al + 21] = 1.0
    IOTA32 = np.full((128, 1), SENTINEL, f32)
    for p in range(128):
        if p % 32 <= 20:
            IOTA32[p, 0] = (p % 32) - 1.0

    FOLD = np.concatenate([np.eye(HID, dtype=f32)] * 2, axis=0)  # [128, 64]

    # uniform per-(a,h) surplus accumulated into H:
    #   2M (A-pass max offset) + M (B-pass) + sum_{j in J_DVE} (M - w_jh)
    SwD = W2act[J_DVE].sum(axis=0)
    extra_h = 2 * M + M + (M * len(J_DVE) - SwD)
    crw = -(A * extra_h) @ fc3_w

    FC3AUG = np.zeros((128, NA), f32)
    FC3AUG[0:64] = fc3_w
    FC3AUG[64] = np.asarray(fc3_b, f32)
    FC3AUG[96] = crw

    return {
        "W1A": W1[0:128].copy(), "W1B": W1[128:210].copy(),
        "B1T": B1.reshape(5, 128).T.copy(),
        "W2C0": W2[0:128].copy(), "W2C1": W2[128:256].copy(),
        "W2C2": W2[256:384].copy(), "W2C3": W2[384:512].copy(),
        "W2C4": W2[512:640].copy(),
        "B2T": np.pad(B2, (0, 64)).reshape(2, 128).T.copy(),
        "FC1A": np.asarray(fc1_w, f32)[0:128].copy(),
        "FC1B": np.asarray(fc1_w, f32)[128:192].copy(),
        "FB1": np.asarray(fc1_b, f32).reshape(64, 1).copy(),
        "W2X": W2x.copy(), "W2F": W2f.copy(),
        "FB2": np.asarray(fc2_b, f32).reshape(64, 1).copy(),
        "LOH_AB": LOH_AB, "LAC_AB": LAC_AB, "LB_AB": LB_AB,
        "LOH_E": LOH_E, "LAC_E": LAC_E, "LB_E": LB_E,
        "SCAL_AB": SCAL_AB, "NWA": NWA, "WAB": WAB,
        "REPL": REPL, "IOTA32": IOTA32, "FOLD": FOLD, "FC3AUG": FC3AUG,
    }


CONST_SHAPES = {
    "W1A": (128, 640), "W1B": (82, 640), "B1T": (128, 5),
    "W2C0": (128, 192), "W2C1": (128, 192), "W2C2": (128, 192),
    "W2C3": (128, 192), "W2C4": (128, 192), "B2T": (128, 2),
    "FC1A": (128, 64), "FC1B": (64, 64), "FB1": (64, 1),
    "W2X": (64, 64), "W2F": (32, 64), "FB2": (64, 1),
    "LOH_AB": (128, 128), "LAC_AB": (128, 128), "LB_AB": (64, 128),
    "LOH_E": (128, 64), "LAC_E": (128, 64), "LB_E": (64, 64),
    "SCAL_AB": (128, 1), "NWA": (128, 21), "WAB": (128, 21),
    "REPL": (128, 128), "IOTA32": (128, 1), "FOLD": (128, 64),
    "FC3AUG": (128, 21),
}


def _build_bass(loop_n=None):
    from contextlib import ExitStack
    import concourse.bass as bass
    import concourse.bacc as bacc
    import concourse.mybir as mybir
    from concourse.tile import TileContext
    from concourse.masks import make_identity
    from concourse.alu_op_type import AluOpType as Op

    f32 = mybir.dt.float32
    Relu = mybir.ActivationFunctionType.Relu
    Ident = mybir.ActivationFunctionType.Identity
    nc = bacc.Bacc("TRN2")

    obs_t = nc.dram_tensor("obs", [R, OBS_D], f32, kind="ExternalInput")
    out_t = nc.dram_tensor("out", [R, NA], f32, kind="ExternalOutput")
    c_t = {k: nc.dram_tensor(k, list(v), f32, kind="ExternalInput")
           for k, v in CONST_SHAPES.items()}

    with TileContext(nc) as tc, ExitStack() as ctx:
        cpool = ctx.enter_context(tc.tile_pool(name="consts", bufs=1))
        dpool = ctx.enter_context(tc.tile_pool(name="dram", bufs=1, space="DRAM"))
        spool = ctx.enter_context(tc.tile_pool(name="sbuf", bufs=1))
        wpool = ctx.enter_context(tc.tile_pool(name="work", bufs=3))
        ppool = ctx.enter_context(tc.tile_pool(name="psum", bufs=3, space="PSUM"))
        hpool = ctx.enter_context(tc.tile_pool(name="hold", bufs=1, space="PSUM"))

        C = {}
        for k, shp in CONST_SHAPES.items():
            C[k] = cpool.tile(list(shp), f32, name=f"c_{k}", tag=f"c_{k}")
            nc.sync.dma_start(C[k][:, :], c_t[k].ap())
        ident = cpool.tile([128, 128], f32, name="ident", tag="ident")
        make_identity(nc, ident[:, :])

        if loop_n is not None:
            ctx.enter_context(tc.For_i(0, loop_n, 1))

        OB = []    # view+feat cols 0:242
        OB32 = []  # nb cols 242:682, each 22-col a-block padded to 32
        for i in range(4):
            rows = obs_t.ap()[i * 128:(i + 1) * 128, :]
            ob = spool.tile([128, 242], f32, name=f"ob{i}", tag=f"ob{i}")
            nc.sync.dma_start(ob[:, :], rows[:, 0:242])
            ob32 = spool.tile([128, 640], f32, name=f"ob32_{i}", tag=f"ob32_{i}")
            dst = bass.AP(ob32[:, :].tensor, 0, [[640, 128], [32, 20], [1, 22]])
            nc.sync.dma_start(dst, rows[:, 242:OBS_D])
            OB32.append(ob32)
            OB.append(ob)

        def transpose_fam(in_aps, rows_out, tag, copy_engine):
            ps = ppool.tile([rows_out, 512], f32, name="ps", tag="ps")
            for i in range(4):
                nc.tensor.transpose(ps[:, i * 128:(i + 1) * 128], in_aps[i],
                                    ident[:, :])
            dst = spool.tile([rows_out, 512], f32, name=f"t_{tag}", tag=f"t_{tag}")
            if copy_engine == "vector":
                nc.vector.tensor_copy(out=dst[:, :], in_=ps[:, :])
            else:
                nc.scalar.copy(dst[:, :], ps[:, :])
            return dst

        VT0 = transpose_fam([ob[:, 0:128] for ob in OB], 128, "vt0", "scalar")
        VT1 = transpose_fam([ob[:, 128:210] for ob in OB], 82, "vt1", "scalar")
        FT = transpose_fam([ob[:, 210:242] for ob in OB], 32, "ft", "scalar")

        # neighbor cols in 32-padded per-a blocks:
        # NBT32[q][32*a4 + e, b] = obs[b, 242 + 22*(4q + a4) + e]
        NBT32 = []
        for q in range(5):
            in_aps = [ob32[:, 128 * q:128 * (q + 1)] for ob32 in OB32]
            NBT32.append(transpose_fam(in_aps, 128, f"nbt32_{q}",
                                       "vector" if q % 2 else "scalar"))

        # one-hot tiles: OHT[q][32*a4 + i, b] = (ids[b, 4q+a4] == i-1)
        OHT = []
        for q in range(5):
            ps = ppool.tile([128, 512], f32, name="ps", tag="ps")
            nc.tensor.matmul(ps[:, :], C["REPL"][:, :], NBT32[q][:, :],
                             start=True, stop=True)
            d = spool.tile([128, 512], f32, name=f"oht{q}", tag=f"oht{q}")
            nc.vector.tensor_single_scalar(
                out=d[:, :], in_=ps[:, :], scalar=C["IOTA32"][:, 0:1],
                op=Op.is_equal)
            OHT.append(d)

        # ---- conv1 ----
        A1 = []
        for m in range(5):
            ps = ppool.tile([128, 512], f32, name="ps", tag="ps")
            nc.tensor.matmul(ps[:, :], C["W1A"][:, m * 128:(m + 1) * 128],
                             VT0[:, :], start=True, stop=False)
            nc.tensor.matmul(ps[:, :], C["W1B"][:, m * 128:(m + 1) * 128],
                             VT1[:, :], start=False, stop=True)
            d = spool.tile([128, 512], f32, name=f"a1_{m}", tag=f"a1_{m}")
            nc.scalar.activation(d[:, :], ps[:, :], Relu,
                                 bias=C["B1T"][:, m:m + 1])
            A1.append(d)

        # ---- conv2 ----
        A2 = []
        for m in range(2):
            mw = 128 if m == 0 else 64
            ps = ppool.tile([mw, 512], f32, name="ps", tag="ps")
            for k in range(5):
                nc.tensor.matmul(ps[:, :],
                                 C[f"W2C{k}"][:, m * 128:m * 128 + mw],
                                 A1[k][:, :], start=(k == 0), stop=(k == 4))
            d = spool.tile([mw, 512], f32, name=f"a2_{m}", tag=f"a2_{m}")
            nc.scalar.activation(d[:, :], ps[:, :], Relu,
                                 bias=C["B2T"][0:mw, m:m + 1])
            A2.append(d)

        # ---- fc1 ----
        ps = ppool.tile([64, 512], f32, name="ps", tag="ps")
        nc.tensor.matmul(ps[:, :], C["FC1A"][:, :], A2[0][:, :],
                         start=True, stop=False)
        nc.tensor.matmul(ps[:, :], C["FC1B"][:, :], A2[1][:, :],
                         start=False, stop=True)
        XT = spool.tile([64, 512], f32, name="xt", tag="xt")
        nc.scalar.activation(XT[:, :], ps[:, :], Relu, bias=C["FB1"][:, 0:1])

        # ---- base ----
        ps = ppool.tile([64, 512], f32, name="ps", tag="ps")
        nc.tensor.matmul(ps[:, :], C["W2X"][:, :], XT[:, :],
                         start=True, stop=False)
        nc.tensor.matmul(ps[:, :], C["W2F"][:, :], FT[:, :],
                         start=False, stop=True)
        BASET = spool.tile([64, 512], f32, name="baset", tag="baset")
        nc.scalar.activation(BASET[:, :], ps[:, :], Ident,
                             bias=C["FB2"][:, 0:1])

        # ---- per-b counts: n = sum_a (e + 21u - 20uv) ----
        NCOLS = spool.tile([128, 4], f32, name="ncols", tag="ncols")
        for i in range(4):
            nbm = OB32[i][:, :].rearrange("p (a e) -> p a e", e=32)
            ids = nbm[:, :, 0]
            acts = nbm[:, :, 1:22]
            e = wpool.tile([128, 20], f32, name="cnt_e", tag="cnt_e")
            u = wpool.tile([128, 20], f32, name="cnt_u", tag="cnt_u")
            v = wpool.tile([128, 20], f32, name="cnt_v", tag="cnt_v")
            nc.vector.tensor_single_scalar(out=e[:, :], in_=ids, scalar=-1.0,
                                           op=Op.is_equal)
            nc.vector.tensor_single_scalar(out=u[:, :], in_=ids, scalar=-0.5,
                                           op=Op.is_ge)
            nc.vector.tensor_reduce(out=v[:, :], in_=acts,
                                    axis=mybir.AxisListType.X, op=Op.add)
            uv = wpool.tile([128, 20], f32, name="cnt_uv", tag="cnt_uv")
            nc.vector.tensor_mul(out=uv[:, :], in0=u[:, :], in1=v[:, :])
            t2 = wpool.tile([128, 20], f32, name="cnt_t2", tag="cnt_t2")
            nc.vector.scalar_tensor_tensor(
                out=t2[:, :], in0=u[:, :], scalar=21.0, in1=e[:, :],
                op0=Op.mult, op1=Op.add)
            t3 = wpool.tile([128, 20], f32, name="cnt_t3", tag="cnt_t3")
            nc.vector.scalar_tensor_tensor(
                out=t3[:, :], in0=uv[:, :], scalar=-20.0, in1=t2[:, :],
                op0=Op.mult, op1=Op.add)
            nc.vector.tensor_reduce(out=NCOLS[:, i:i + 1], in_=t3[:, :],
                                    axis=mybir.AxisListType.X, op=Op.add)

        # ---- neighbor phase ----
        ACCAB = spool.tile([128, 512], f32, name="accab", tag="accab")
        ACCE = spool.tile([128, 512], f32, name="acce", tag="acce")
        nc.vector.memset(ACCAB[:, :], 0.0)
        nc.vector.memset(ACCE[:, :], 0.0)

        HPS = hpool.tile([64, 512], f32, name="hps", tag="hps")
        n_hmm = [0]
        NHMM_TOT = len(J_ACT) * (A // 2) + 2

        def hacc(tile_ap):
            nc.tensor.matmul(HPS[:, :], C["FOLD"][:, :], tile_ap,
                             start=(n_hmm[0] == 0),
                             stop=(n_hmm[0] == NHMM_TOT - 1),
                             skip_group_check=True)
            n_hmm[0] += 1

        for apair in range(A // 2):
            PCE = ppool.tile([128, 512], f32, name="ps", tag="ps")
            for half in range(2):
                a = 2 * apair + half
                q, a4 = a // 4, a % 4
                r = 32 * a4
                po = PCE[64 * half:64 * half + 64, :]
                nc.tensor.matmul(po, C["LOH_E"][r:r + 21, :],
                                 OHT[q][r:r + 21, :], start=True, stop=False,
                                 skip_group_check=True,
                                 tile_position=(r, 64 * half))
                nc.tensor.matmul(po, C["LAC_E"][r:r + 22, :],
                                 NBT32[q][r:r + 22, :], start=False,
                                 stop=False, skip_group_check=True,
                                 tile_position=(r, 64 * half))
                nc.tensor.matmul(po, C["LB_E"][:, :], BASET[:, :],
                                 start=False, stop=True, skip_group_check=True,
                                 tile_position=(0, 64 * half))
            CE = wpool.tile([128, 512], f32, name="ce", tag="ce")
            nc.scalar.copy(CE[:, :], PCE[:, :])
            for j in J_DVE:
                nc.vector.scalar_tensor_tensor(
                    out=ACCE[:, :], in0=CE[:, :], scalar=C["NWA"][:, j:j + 1],
                    in1=ACCE[:, :], op0=Op.max, op1=Op.add)
            for j in J_ACT:
                rj = wpool.tile([128, 512], f32, name="relu_j", tag="relu_j")
                nc.scalar.activation(rj[:, :], CE[:, :], Relu,
                                     bias=C["WAB"][:, j:j + 1])
                hacc(rj[:, :])

        for a in range(A):
            q, a4 = a // 4, a % 4
            r = 32 * a4
            PAB = ppool.tile([128, 512], f32, name="ps", tag="ps")
            nc.tensor.matmul(PAB[:, :], C["LOH_AB"][r:r + 21, :],
                             OHT[q][r:r + 21, :], start=True, stop=False,
                             skip_group_check=True, tile_position=(r, 0))
            nc.tensor.matmul(PAB[:, :], C["LAC_AB"][r:r + 22, :],
                             NBT32[q][r:r + 22, :], start=False, stop=False,
                             skip_group_check=True, tile_position=(r, 0))
            nc.tensor.matmul(PAB[:, :], C["LB_AB"][:, :], BASET[:, :],
                             start=False, stop=True, skip_group_check=True)
            nc.vector.scalar_tensor_tensor(
                out=ACCAB[:, :], in0=PAB[:, :], scalar=C["SCAL_AB"][:, 0:1],
                in1=ACCAB[:, :], op0=Op.max, op1=Op.add)

        hacc(ACCAB[:, :])
        hacc(ACCE[:, :])
        assert n_hmm[0] == NHMM_TOT

        # ---- assemble HnT [128, 512] (rows 0:64 H, 64 n, 96 ones) ----
        HNT = spool.tile([128, 512], f32, name="hnt", tag="hnt")
        nc.vector.memset(HNT[:, :], 0.0)
        nc.scalar.copy(HNT[0:64, :], HPS[:, :])
        nc.vector.memset(HNT[96:97, :], 1.0)
        # n row via DRAM round-trip: NCOLS [128b, 4i] -> dram [4i, 128b]
        nrow_d = dpool.tile([4, 128], f32, name="nrow_d", tag="nrow_d")
        nc.sync.dma_start(nrow_d[:, :].transpose([1, 0]), NCOLS[:, :])
        nc.sync.dma_start(HNT[64:65, :],
                          nrow_d[:, :].rearrange("i b -> (i b)").unsqueeze(0))

        # ---- final out ----
        for i in range(4):
            ps = ppool.tile([128, NA], f32, name="ps", tag="ps")
            nc.tensor.matmul(ps[:, :], HNT[:, i * 128:(i + 1) * 128],
                             C["FC3AUG"][:, :], start=True, stop=True)
            d = wpool.tile([128, NA], f32, name="out_sb", tag="out_sb")
            nc.scalar.copy(d[:, :], ps[:, :])
            nc.sync.dma_start(out_t.ap()[i * 128:(i + 1) * 128, :], d[:, :])

    nc.compile()
    return nc


def _get_nc():
    if "nc" not in _CACHE:
        _CACHE["nc"] = _build_bass()
    return _CACHE["nc"]


def kernel(**inputs) -> np.ndarray:
    in_maps = _in_maps(inputs)
    if "r1" not in _CACHE:
        _CACHE["r1"] = _make_runner(_get_nc())
    res, _ = _CACHE["r1"](in_maps)
    return np.ascontiguousarray(res["out"])


def _make_runner(nc):
    """Compile nc for 8 cores once; return f(in_maps, n_rep) -> best seconds."""
    import time
    import jax
    import numpy as np
    from jax.sharding import Mesh, PartitionSpec
    from jax.experimental.shard_map import shard_map
    import concourse.mybir as mybir
    from concourse import bass2jax

    bass2jax.install_neuronx_cc_hook()
    partition_name = nc.partition_id_tensor.name if nc.partition_id_tensor else None
    in_names, out_names, out_avals, zero_outs = [], [], [], []
    for alloc in nc.m.functions[0].allocations:
        if not isinstance(alloc, mybir.MemoryLocationSet):
            continue
        name = alloc.memorylocations[0].name
        if alloc.kind == "ExternalInput":
            if name != partition_name:
                in_names.append(name)
        elif alloc.kind == "ExternalOutput":
            shape = tuple(alloc.tensor_shape)
            dtype = mybir.dt.np(alloc.dtype)
            out_names.append(name)
            out_avals.append(jax.core.ShapedArray(shape, dtype))
            zero_outs.append(np.zeros(shape, dtype))
    n_params = len(in_names)
    all_names = in_names + out_names
    if partition_name is not None:
        all_names.append(partition_name)

    def _body(*args):
        operands = list(args)
        if partition_name is not None:
            operands.append(bass2jax.partition_id_tensor())
        return tuple(bass2jax._bass_exec_p.bind(
            *operands, out_avals=tuple(out_avals), in_names=tuple(all_names),
            out_names=tuple(out_names), lowering_input_output_aliases=(),
            sim_require_finite=False, sim_require_nnan=False, nc=nc))

    devices = jax.devices()[:NCORES]
    mesh = Mesh(np.asarray(devices), ("core",))
    nio = n_params + len(out_names)
    sharded = jax.jit(
        shard_map(_body, mesh=mesh,
                  in_specs=(PartitionSpec("core"),) * nio,
                  out_specs=(PartitionSpec("core"),) * len(out_names),
                  check_rep=False),
        keep_unused=True)

    def run(in_maps, n_rep=1, timed=False):
        concat_in = [np.concatenate([np.asarray(in_maps[c][k])
                                     for c in range(NCORES)], axis=0)
                     for k in in_names]
        concat_zero = [np.zeros((NCORES * z.shape[0], *z.shape[1:]), z.dtype)
                       for z in zero_outs]
        dev_args = [jax.device_put(a) for a in concat_in + concat_zero]
        outs = sharded(*dev_args)
        jax.block_until_ready(outs)
        best = None
        if timed:
            for _ in range(n_rep):
                t0 = time.perf_counter()
                outs = sharded(*dev_args)
                jax.block_until_ready(outs)
                dt = time.perf_counter() - t0
                best = dt if best is None else min(best, dt)
        res = {name: np.asarray(outs[i]) for i, name in enumerate(out_names)}
        return res, best

    return run


def _in_maps(inputs):
    obs = np.ascontiguousarray(inputs["obs"], dtype=np.float32)
    consts = _build_consts(
        inputs["conv1_w"], inputs["conv1_b"], inputs["conv2_w"],
        inputs["conv2_b"], inputs["fc1_w"], inputs["fc1_b"],
        inputs["fc2_w"], inputs["fc2_b"], inputs["fc3_w"], inputs["fc3_b"])
    consts = {k: np.ascontiguousarray(v, dtype=np.float32)
              for k, v in consts.items()}
    return [dict(consts, obs=obs[c * R:(c + 1) * R]) for c in range(NCORES)]


LOOP_N = 32


def time_kernel(**inputs):
    """Return estimated per-invocation HW ns via loop-differencing."""
    in_maps = _in_maps(inputs)
    if "r1" not in _CACHE:
        _CACHE["r1"] = _make_runner(_get_nc())
    if "rN" not in _CACHE:
        _CACHE["rN"] = _make_runner(_build_bass(loop_n=LOOP_N))
    _, t1 = _CACHE["r1"](in_maps, n_rep=5, timed=True)
    resN, tN = _CACHE["rN"](in_maps, n_rep=5, timed=True)
    print(f"  t1={t1*1e6:.1f} us  t{LOOP_N}={tN*1e6:.1f} us")
    return (tN - t1) / (LOOP_N - 1) * 1e9
